# revision 19
# baseline (speedup 1.0000x reference)
"""Trainium2 Bass kernel for nn_Net_Actor (7-layer GAT stack + GRU head).

Sharding: 8 cores, core c owns dst nodes [4000c, 4000c+4000). Edges (incl.
self-loops) are dst-sorted per core and cut into 128-edge tiles aligned to
128-dst windows (tile counts equalized across cores so the SPMD program is
identical). Per edge tile:
  - a2|h rows (bf16, 512B) are fetched with one dma_gather (src-indexed)
  - a1[dst] is expanded from the window's contiguous a1 rows by a one-hot
    matmul (St, host-built), and a2 is added via an identity matmul into the
    same PSUM tile -> pre-activation a
  - t = exp(lrelu(a)) (no max subtraction; logits are small)
  - num|den segment sums via a one-hot matmul (S, built on-device) into a
    per-window PSUM accumulator
Between layers each core builds its table shard with one matmul per window
(folded weights) and an AllGather replicates the table. The GRU branch and
output heads run on host (0.02% of FLOPs).
"""
import sys

sys.path.insert(0, "/opt/trn_rl_repo")

import numpy as np
import ml_dtypes

import concourse.bacc as bacc
import concourse.mybir as mybir
import concourse.tile as tile
from concourse.bass_utils import run_bass_kernel_spmd

BF16 = ml_dtypes.bfloat16

NCORES = 8
ND = 128
N = 32000
NLOC = N // NCORES          # 4000
NLOCP = 4096                # padded a1 table rows
NW = (NLOC + 127) // 128    # 32 windows; last one covers 32 dst
WLEN = [128] * (NW - 1) + [NLOC - 128 * (NW - 1)]
B = 64
NN = 500
EMB = 128
C = 16                      # tiles per gather chunk
LRELU_VIA_DVE = False       # sim-compatible lrelu (max(x, 0.2x)); HW uses ACT
NLAYERS = 7                 # debug knob: run only the first NLAYERS layers
TRACE = False               # capture NTFF profile (sets exec_time_ns)

LAYER_SET = [0, 1, 1, 0, 2, 2, 0]
LAYER_PAR = ["g0", "g1", "g1", "gn", "g2", "g2", "gn"]


# ---------------------------------------------------------------- host prep

def _prep_edge_set(ei):
    """ei: [2, E'] int array (global). Per-core dst-sorted tile streams with
    window tile counts T_w equalized across cores. Returns per core:
    (src_ids int16, slot f32 (-1 padding)), plus T_w, NT."""
    src = np.concatenate([np.asarray(ei[0], np.int64), np.arange(N, dtype=np.int64)])
    dst = np.concatenate([np.asarray(ei[1], np.int64), np.arange(N, dtype=np.int64)])
    order = np.argsort(dst, kind="stable")
    src, dst = src[order], dst[order]

    lo = np.searchsorted(dst, np.arange(NCORES) * NLOC)
    hi = np.searchsorted(dst, (np.arange(NCORES) + 1) * NLOC)

    counts = np.zeros((NCORES, NW), np.int64)
    per_core = []
    for c in range(NCORES):
        s_c = src[lo[c]:hi[c]]
        d_c = dst[lo[c]:hi[c]] - c * NLOC
        counts[c] = np.bincount(d_c >> 7, minlength=NW)
        per_core.append((s_c, d_c))

    T_w = np.maximum(1, (counts.max(axis=0) + 127) // 128)
    NT = int(T_w.sum())

    out = []
    for c in range(NCORES):
        s_c, d_c = per_core[c]
        src_ids = np.zeros(NT * 128, np.int16)
        slot = np.full(NT * 128, -1.0, np.float32)
        off_e = 0
        off_t = 0
        for w in range(NW):
            ne = int(counts[c, w])
            sl = slice(off_t * 128, off_t * 128 + ne)
            src_ids[sl] = s_c[off_e:off_e + ne]
            slot[sl] = (d_c[off_e:off_e + ne] - 128 * w).astype(np.float32)
            off_e += ne
            off_t += int(T_w[w])
        out.append((src_ids, slot))
    return out, T_w.astype(np.int64), NT


def _wrap_idx(idx):
    """[NT*128] -> [128, NT*8] int16: idx i at [i%16, i//16], replicated x8."""
    w = idx.reshape(-1, 16).T
    return np.tile(w, (8, 1)).astype(np.int16)


def _slot_T(slot, NT):
    """[NT*128] -> [128, NT]: edge e=(t*128+p) -> [p, t]."""
    return slot.reshape(NT, 128).T.copy()


def _make_St(slot, NT):
    """Transposed one-hots: St[j, t*128+e] = (slot[t*128+e] == j), bf16."""
    sl = slot.reshape(NT * 128).astype(np.int32)
    St = np.zeros((128, NT * 128), BF16)
    e = np.arange(NT * 128)
    m = sl >= 0
    St[sl[m], e[m]] = 1.0
    return St


def _fold(lw, lb, aw, ab):
    awi, awj = aw[:, :ND], aw[:, ND:]
    Wh = lw.T
    Wa1 = lw.T @ awi.T
    Wa2 = lw.T @ awj.T
    b1 = lb @ awi.T + ab
    b2 = lb @ awj.T
    return Wh, Wa1, Wa2, lb, b1, b2


# ------------------------------------------------------------- bass builder

_BUILD_CACHE = {}


def _build(T_w_sets, NT_sets):
    nc = bacc.Bacc("TRN2", target_bir_lowering=False, debug=False,
                   num_devices=NCORES)
    f32, bf16, i16 = mybir.dt.float32, mybir.dt.bfloat16, mybir.dt.int16

    d_tab0 = nc.dram_tensor("tab0", [N, 256], bf16, kind="ExternalInput")
    d_a1t0 = nc.dram_tensor("a1t0", [NLOCP, ND], bf16, kind="ExternalInput")
    d_iota = nc.dram_tensor("iota", [128, 128], bf16, kind="ExternalInput")
    d_ident = nc.dram_tensor("ident", [128, 128], f32, kind="ExternalInput")
    d_identb = nc.dram_tensor("identb", [128, 128], bf16, kind="ExternalInput")
    d_W = nc.dram_tensor("W", [6, 128, 384], bf16, kind="ExternalInput")
    d_bt = nc.dram_tensor("bt", [6, 128, 384], f32, kind="ExternalInput")
    d_srcw, d_slot, d_St = [], [], []
    for s in range(3):
        NT = NT_sets[s]
        d_srcw.append(nc.dram_tensor(f"srcw{s}", [128, NT * 8], i16,
                                     kind="ExternalInput"))
        d_slot.append(nc.dram_tensor(f"slot{s}", [128, NT], f32,
                                     kind="ExternalInput"))
        d_St.append(nc.dram_tensor(f"St{s}", [128, NT * 128], bf16,
                                   kind="ExternalInput"))
    d_gx = nc.dram_tensor("gx", [NLOC, ND], f32, kind="ExternalOutput")

    with tile.TileContext(nc) as tc:
        with (
            tc.tile_pool(name="const", bufs=1) as constp,
            tc.tile_pool(name="idx", bufs=1) as idxp,
            tc.tile_pool(name="resid", bufs=1) as residp,
            tc.tile_pool(name="chunk", bufs=2) as chunkp,
            tc.tile_pool(name="win", bufs=2) as winp,
            tc.tile_pool(name="a1w", bufs=4) as a1wp,
            tc.tile_pool(name="pnd", bufs=2, space="PSUM") as pndp,
            tc.tile_pool(name="pa", bufs=3, space="PSUM") as pap,
            tc.tile_pool(name="pmisc", bufs=2, space="PSUM") as pmiscp,
            tc.tile_pool(name="dram", bufs=2, space="DRAM") as dramp,
        ):
            t_iota = constp.tile([128, 128], bf16)
            t_ident = constp.tile([128, 128], f32)
            t_identb = constp.tile([128, 128], bf16)
            t_W = constp.tile([128, 6, 384], bf16)
            t_bt = constp.tile([128, 6, 384], f32)
            nc.sync.dma_start(t_iota[:], d_iota[:])
            nc.sync.dma_start(t_ident[:], d_ident[:])
            nc.sync.dma_start(t_identb[:], d_identb[:])
            for l in range(6):
                nc.sync.dma_start(t_W[:, l, :], d_W[l])
                nc.sync.dma_start(t_bt[:, l, :], d_bt[l])

            t_srcw, t_slot = [], []
            for s in range(3):
                NT = NT_sets[s]
                a = idxp.tile([128, NT * 8], i16, tag=f"srcw{s}")
                c_ = idxp.tile([128, NT], f32, tag=f"slot{s}")
                nc.sync.dma_start(a[:], d_srcw[s][:])
                nc.sync.dma_start(c_[:], d_slot[s][:])
                t_srcw.append(a)
                t_slot.append(c_)

            t_g0 = residp.tile([128, NW, 128], bf16, tag="g0")
            t_g3 = residp.tile([128, NW, 128], bf16, tag="g3")

            tab_prev = None
            a1_prev = None

            last = NLAYERS - 1
            for l in range(NLAYERS):
                s = LAYER_SET[l]
                NT = NT_sets[s]
                T_w = T_w_sets[s]
                nchunks = (NT + C - 1) // C

                if l == 0:
                    tab_ap = d_tab0[:]
                    a1_dram = d_a1t0
                else:
                    tab_ap = tab_prev[:].rearrange("r n c -> (r n) c")
                    a1_dram = a1_prev

                if l < last:
                    b_agin = dramp.tile([NLOC, 256], bf16, tag="agin")
                    b_agout = dramp.tile([NCORES, NLOC, 256], bf16,
                                         tag="agout", addr_space="Shared")
                    b_a1n = dramp.tile([NLOCP, ND], bf16, tag="a1n")

                chunk_tiles = [None] * nchunks
                tile_win = []
                for w in range(NW):
                    tile_win += [w] * int(T_w[w])
                a1w_tiles = {}

                def get_a1w(w, a1_dram=a1_dram):
                    if w not in a1w_tiles:
                        t = a1wp.tile([128, 128], bf16, tag="t_a1w")
                        nc.sync.dma_start(t[:],
                                          a1_dram[w * 128:w * 128 + 128, :])
                        a1w_tiles[w] = t
                    return a1w_tiles[w]

                def make_chunk(k, s=s, NT=NT, tab_ap=tab_ap):
                    ck = min(C, NT - k * C)
                    ne = ck * 128
                    g_eh = chunkp.tile([128, C, 256], bf16, tag="g_eh")
                    t_St = chunkp.tile([128, C, 128], bf16, tag="t_St")
                    t_S = chunkp.tile([128, C, 128], bf16, tag="t_S")
                    t_e1 = chunkp.tile([128, C, 128], bf16, tag="t_e1")
                    t_e2 = chunkp.tile([128, C, 128], bf16, tag="t_e2")
                    t_rhs = chunkp.tile([128, C, 256], bf16, tag="t_rhs")
                    isl = slice(k * C * 8, (k * C + ck) * 8)
                    nc.gpsimd.dma_gather(g_eh[:, :ck, :], tab_ap,
                                         t_srcw[s][:, isl], ne, ne, 256,
                                         single_packet=False)
                    nc.sync.dma_start(t_St[:, :ck, :],
                                      d_St[s][:, k * C * 128:
                                              (k * C + ck) * 128]
                                      .rearrange("p (t e) -> p t e", e=128))
                    # S[e, tile, j] = (iota[e, j] == slot[e, tile])
                    nc.vector.tensor_tensor(
                        t_S[:, :ck, :],
                        t_iota[:, None, :].broadcast_to([128, ck, 128]),
                        t_slot[s][:, k * C:k * C + ck, None]
                        .broadcast_to([128, ck, 128]),
                        mybir.AluOpType.is_equal)
                    for j in range(ck):
                        # pre-activation a = St.T @ a1_win + a2 (PSUM)
                        ta1w = get_a1w(tile_win[k * C + j])
                        p_a = pap.tile([128, 128], f32, tag="p_a")
                        nc.tensor.matmul(p_a[:], t_St[:, j, :], ta1w[:],
                                         start=True, stop=False)
                        nc.tensor.matmul(p_a[:], t_identb[:],
                                         g_eh[:, j, 0:128],
                                         start=False, stop=True)
                        # t = exp(lrelu(a)) = max(exp(a), exp(0.2*a)):
                        # exp-only ACT usage -> no activation-table thrash
                        nc.scalar.activation(t_e1[:, j, :], p_a[:],
                                             mybir.ActivationFunctionType.Exp)
                        nc.scalar.activation(t_e2[:, j, :], p_a[:],
                                             mybir.ActivationFunctionType.Exp,
                                             scale=0.2)
                    nc.vector.tensor_tensor(t_rhs[:, :ck, 128:256],
                                            t_e1[:, :ck, :],
                                            t_e2[:, :ck, :],
                                            mybir.AluOpType.max)
                    nc.vector.tensor_tensor(t_rhs[:, :ck, 0:128],
                                            t_rhs[:, :ck, 128:256],
                                            g_eh[:, :ck, 128:256],
                                            mybir.AluOpType.mult)
                    return g_eh, t_St, t_S, t_e1, t_e2, t_rhs

                tg = 0
                for w in range(NW):
                    wl = WLEN[w]
                    p_nd = pndp.tile([128, 256], f32, tag="p_nd")
                    for i in range(int(T_w[w])):
                        k, j = divmod(tg, C)
                        if chunk_tiles[k] is None:
                            chunk_tiles[k] = make_chunk(k)
                        g_eh, t_St, t_S, t_e1, t_e2, t_rhs = chunk_tiles[k]
                        nc.tensor.matmul(p_nd[:], t_S[:, j, :], t_rhs[:, j, :],
                                         start=(i == 0),
                                         stop=(i == int(T_w[w]) - 1))
                        tg += 1

                    # window epilogue: x' = num * recip(den+eps) (+ residual)
                    t_den = winp.tile([128, 128], f32, tag="t_den")
                    t_rw = winp.tile([128, 128], f32, tag="t_rw")
                    t_xw = winp.tile([128, 128], f32, tag="t_xw")
                    nc.vector.tensor_scalar(t_den[:], p_nd[:, 128:256], 1e-16,
                                            None, mybir.AluOpType.add)
                    nc.vector.reciprocal(t_rw[:], t_den[:])
                    if l == 3:
                        t_x0 = winp.tile([128, 128], f32, tag="t_x0")
                        nc.vector.tensor_tensor(t_x0[:], p_nd[:, 0:128],
                                                t_rw[:], mybir.AluOpType.mult)
                        nc.vector.tensor_tensor(t_xw[:], t_x0[:],
                                                t_g0[:, w, :],
                                                mybir.AluOpType.add)
                    elif l == 6:
                        t_x0 = winp.tile([128, 128], f32, tag="t_x0")
                        nc.vector.tensor_tensor(t_x0[:], p_nd[:, 0:128],
                                                t_rw[:], mybir.AluOpType.mult)
                        nc.vector.tensor_tensor(t_xw[:], t_x0[:],
                                                t_g3[:, w, :],
                                                mybir.AluOpType.add)
                    else:
                        nc.vector.tensor_tensor(t_xw[:], p_nd[:, 0:128],
                                                t_rw[:], mybir.AluOpType.mult)
                    if l == 0:
                        nc.vector.tensor_copy(t_g0[:, w, :], t_xw[:])
                    if l == 3:
                        nc.vector.tensor_copy(t_g3[:, w, :], t_xw[:])

                    if l == last:
                        nc.sync.dma_start(d_gx[w * 128:w * 128 + wl, :],
                                          t_xw[:wl, :])
                    else:
                        # transpose -> table matmul -> bias/split -> bounce
                        p_tr = pmiscp.tile([128, 384], f32, tag="pm")
                        nc.tensor.transpose(p_tr[:, 0:128], t_xw[:],
                                            t_ident[:])
                        t_xT = winp.tile([128, 128], bf16, tag="t_xT")
                        nc.vector.tensor_copy(t_xT[:], p_tr[:, 0:128])
                        p_tab = pmiscp.tile([128, 384], f32, tag="pm")
                        nc.tensor.matmul(p_tab[:], t_xT[:], t_W[:, l, :],
                                         start=True, stop=True)
                        t_tb = winp.tile([128, 256], bf16, tag="t_tb")
                        t_a1o = winp.tile([128, 128], bf16, tag="t_a1o")
                        nc.vector.tensor_tensor(t_tb[:], p_tab[:, 0:256],
                                                t_bt[:, l, 0:256],
                                                mybir.AluOpType.add)
                        nc.vector.tensor_tensor(t_a1o[:], p_tab[:, 256:384],
                                                t_bt[:, l, 256:384],
                                                mybir.AluOpType.add)
                        nc.sync.dma_start(b_agin[w * 128:w * 128 + wl, :],
                                          t_tb[:wl, :])
                        nc.sync.dma_start(b_a1n[w * 128:w * 128 + 128, :],
                                          t_a1o[:])

                if l < last:
                    nc.gpsimd.collective_compute(
                        "AllGather", mybir.AluOpType.bypass,
                        replica_groups=[list(range(NCORES))],
                        ins=[b_agin[:].opt()], outs=[b_agout[:].opt()])
                    tab_prev = b_agout
                    a1_prev = b_a1n

    nc.compile()
    return nc


# ------------------------------------------------------------------ kernel

def kernel(**inputs):
    f32 = np.float32
    x = np.asarray(inputs["x"], f32)
    state_ = np.asarray(inputs["state_"], f32)
    input_ = np.asarray(inputs["input_"], f32)

    par = {k: np.asarray(v, f32) for k, v in inputs.items()
           if k not in ("x", "state_", "input_", "e_n", "e_r0", "e_r1",
                        "n_nodes")}

    sets_raw = [inputs["e_n"], inputs["e_r1"], inputs["e_r0"]]
    prep = [_prep_edge_set(np.asarray(e)) for e in sets_raw]
    T_w_sets = [p[1] for p in prep]
    NT_sets = [p[2] for p in prep]

    folds = []
    for l in range(7):
        p = LAYER_PAR[l]
        folds.append(_fold(par[f"{p}_lw"], par[f"{p}_lb"],
                           par[f"{p}_aw"], par[f"{p}_ab"]))

    W_dev = np.zeros((6, 128, 384), f32)
    bt_dev = np.zeros((6, 128, 384), f32)
    for l in range(1, 7):
        Wh, Wa1, Wa2, bh, b1, b2 = folds[l]
        W_dev[l - 1] = np.concatenate([Wa2, Wh, Wa1], axis=1)
        bt_dev[l - 1] = np.tile(np.concatenate([b2, bh, b1])[None, :],
                                (128, 1))

    Wh, Wa1, Wa2, bh, b1, b2 = folds[0]
    tab0 = np.concatenate([x @ Wa2 + b2, x @ Wh + bh], axis=1).astype(BF16)
    a1_0 = (x @ Wa1 + b1).astype(BF16)

    iota = np.tile(np.arange(128, dtype=f32), (128, 1)).astype(BF16)
    ident = np.eye(128, dtype=f32)

    meta_key = (NLAYERS, LRELU_VIA_DVE) + tuple(tuple(t) for t in T_w_sets)
    if meta_key not in _BUILD_CACHE:
        _BUILD_CACHE.clear()
        _BUILD_CACHE[meta_key] = _build(T_w_sets, NT_sets)
    nc = _BUILD_CACHE[meta_key]

    in_maps = []
    for c in range(NCORES):
        a1c = np.zeros((NLOCP, ND), BF16)
        a1c[:NLOC] = a1_0[c * NLOC:(c + 1) * NLOC]
        m = {
            "tab0": tab0,
            "a1t0": a1c,
            "iota": iota,
            "ident": ident,
            "identb": ident.astype(BF16),
            "W": W_dev.astype(BF16),
            "bt": bt_dev,
        }
        for s in range(3):
            src_ids, slot = prep[s][0][c]
            m[f"srcw{s}"] = _wrap_idx(src_ids)
            m[f"slot{s}"] = _slot_T(slot, NT_sets[s])
            m[f"St{s}"] = _make_St(slot, NT_sets[s])
        in_maps.append(m)

    res = run_bass_kernel_spmd(nc, in_maps, core_ids=list(range(NCORES)),
                               trace=TRACE)
    kernel.last_result = res
    gx = np.concatenate([res.results[c]["gx"] for c in range(NCORES)], axis=0)

    # ---- host postprocessing: heads ----
    xt = gx.reshape(B, NN, ND)[:, 1:]
    scores = (xt @ par["pw"].T + par["pb"])[..., 0]
    sm = np.exp(scores - scores.max(-1, keepdims=True))
    probs = (sm / sm.sum(-1, keepdims=True)).astype(f32)

    def elu(v):
        return np.where(v > 0, v, np.expm1(np.minimum(v, 0.0))).astype(f32)

    a_out = elu((xt @ par["mw"].T + par["mb"])[..., 0]) + 2
    b_out = elu(np.abs((xt @ par["sw"].T + par["sb"])[..., 0])) + 2

    # ---- host GRU ----
    st = state_.reshape(-1, ND)
    ii = (input_.reshape(st.shape[0], -1, EMB) @ par["win"].T).mean(1)
    si = np.concatenate([st, ii], -1)

    def sig(v):
        return 1.0 / (1.0 + np.exp(-v))

    z = sig(si @ par["wz"].T + par["bz"])
    r = sig(si @ par["wr"].T + par["br"])
    hc = np.tanh(np.concatenate([r * st, ii], -1) @ par["wh"].T + par["bh"])
    h = ((1 - z) * st + z * hc).astype(f32)

    return h, probs, a_out.astype(f32), b_out.astype(f32)


# revision 20
# speedup vs baseline: 1.4031x; 1.4031x over previous
"""Trainium2 Bass kernel for nn_Net_Actor (7-layer GAT stack + GRU head).

Sharding: 8 cores, core c owns dst nodes [4000c, 4000c+4000). Edges (incl.
self-loops) are dst-sorted per core and cut into 128-edge tiles aligned to
128-dst windows (tile counts equalized across cores so the SPMD program is
identical). Per edge tile:
  - a2|h rows (bf16, 512B) are fetched with one dma_gather (src-indexed)
  - a1[dst] is expanded from the window's contiguous a1 rows by a one-hot
    matmul (St, host-built), and a2 is added via an identity matmul into the
    same PSUM tile -> pre-activation a
  - t = exp(lrelu(a)) (no max subtraction; logits are small)
  - num|den segment sums via a one-hot matmul (S, built on-device) into a
    per-window PSUM accumulator
Between layers each core builds its table shard with one matmul per window
(folded weights) and an AllGather replicates the table. The GRU branch and
output heads run on host (0.02% of FLOPs).
"""
import sys

sys.path.insert(0, "/opt/trn_rl_repo")

import numpy as np
import ml_dtypes

import concourse.bacc as bacc
import concourse.mybir as mybir
import concourse.tile as tile
from concourse.bass_utils import run_bass_kernel_spmd

BF16 = ml_dtypes.bfloat16

NCORES = 8
ND = 128
N = 32000
NLOC = N // NCORES          # 4000
NLOCP = 4096                # padded a1 table rows
NW = (NLOC + 127) // 128    # 32 windows; last one covers 32 dst
WLEN = [128] * (NW - 1) + [NLOC - 128 * (NW - 1)]
B = 64
NN = 500
EMB = 128
C = 16                      # tiles per gather chunk
LRELU_VIA_DVE = False       # sim-compatible lrelu (max(x, 0.2x)); HW uses ACT
NLAYERS = 7                 # debug knob: run only the first NLAYERS layers
TRACE = False               # capture NTFF profile (sets exec_time_ns)

LAYER_SET = [0, 1, 1, 0, 2, 2, 0]
LAYER_PAR = ["g0", "g1", "g1", "gn", "g2", "g2", "gn"]


# ---------------------------------------------------------------- host prep

def _prep_edge_set(ei):
    """ei: [2, E'] int array (global). Per-core dst-sorted tile streams with
    window tile counts T_w equalized across cores. Returns per core:
    (src_ids int16, slot f32 (-1 padding)), plus T_w, NT."""
    src = np.concatenate([np.asarray(ei[0], np.int64), np.arange(N, dtype=np.int64)])
    dst = np.concatenate([np.asarray(ei[1], np.int64), np.arange(N, dtype=np.int64)])
    order = np.argsort(dst, kind="stable")
    src, dst = src[order], dst[order]

    lo = np.searchsorted(dst, np.arange(NCORES) * NLOC)
    hi = np.searchsorted(dst, (np.arange(NCORES) + 1) * NLOC)

    counts = np.zeros((NCORES, NW), np.int64)
    per_core = []
    for c in range(NCORES):
        s_c = src[lo[c]:hi[c]]
        d_c = dst[lo[c]:hi[c]] - c * NLOC
        counts[c] = np.bincount(d_c >> 7, minlength=NW)
        per_core.append((s_c, d_c))

    T_w = np.maximum(1, (counts.max(axis=0) + 127) // 128)
    NT = int(T_w.sum())

    out = []
    for c in range(NCORES):
        s_c, d_c = per_core[c]
        src_ids = np.zeros(NT * 128, np.int16)
        slot = np.full(NT * 128, -1.0, np.float32)
        off_e = 0
        off_t = 0
        for w in range(NW):
            ne = int(counts[c, w])
            sl = slice(off_t * 128, off_t * 128 + ne)
            src_ids[sl] = s_c[off_e:off_e + ne]
            slot[sl] = (d_c[off_e:off_e + ne] - 128 * w).astype(np.float32)
            off_e += ne
            off_t += int(T_w[w])
        out.append((src_ids, slot))
    return out, T_w.astype(np.int64), NT


def _wrap_idx(idx):
    """[NT*128] -> [128, NT*8] int16: idx i at [i%16, i//16], replicated x8."""
    w = idx.reshape(-1, 16).T
    return np.tile(w, (8, 1)).astype(np.int16)


def _slot_T(slot, NT):
    """[NT*128] -> [128, NT]: edge e=(t*128+p) -> [p, t]."""
    return slot.reshape(NT, 128).T.copy()


def _make_St(slot, NT):
    """Transposed one-hots: St[j, t*128+e] = (slot[t*128+e] == j), bf16."""
    sl = slot.reshape(NT * 128).astype(np.int32)
    St = np.zeros((128, NT * 128), BF16)
    e = np.arange(NT * 128)
    m = sl >= 0
    St[sl[m], e[m]] = 1.0
    return St


def _fold(lw, lb, aw, ab):
    awi, awj = aw[:, :ND], aw[:, ND:]
    Wh = lw.T
    Wa1 = lw.T @ awi.T
    Wa2 = lw.T @ awj.T
    b1 = lb @ awi.T + ab
    b2 = lb @ awj.T
    return Wh, Wa1, Wa2, lb, b1, b2


# ------------------------------------------------------------- bass builder

_BUILD_CACHE = {}


def _build(T_w_sets, NT_sets):
    nc = bacc.Bacc("TRN2", target_bir_lowering=False, debug=False,
                   num_devices=NCORES)
    f32, bf16, i16 = mybir.dt.float32, mybir.dt.bfloat16, mybir.dt.int16

    d_tab0 = nc.dram_tensor("tab0", [N, 256], bf16, kind="ExternalInput")
    d_a1t0 = nc.dram_tensor("a1t0", [NLOCP, ND], bf16, kind="ExternalInput")
    d_iota = nc.dram_tensor("iota", [128, 128], bf16, kind="ExternalInput")
    d_ident = nc.dram_tensor("ident", [128, 128], f32, kind="ExternalInput")
    d_identb = nc.dram_tensor("identb", [128, 128], bf16, kind="ExternalInput")
    d_W = nc.dram_tensor("W", [6, 128, 384], bf16, kind="ExternalInput")
    d_bt = nc.dram_tensor("bt", [6, 128, 384], f32, kind="ExternalInput")
    d_srcw, d_slot, d_St = [], [], []
    for s in range(3):
        NT = NT_sets[s]
        d_srcw.append(nc.dram_tensor(f"srcw{s}", [128, NT * 8], i16,
                                     kind="ExternalInput"))
        d_slot.append(nc.dram_tensor(f"slot{s}", [128, NT], f32,
                                     kind="ExternalInput"))
        d_St.append(nc.dram_tensor(f"St{s}", [128, NT * 128], bf16,
                                   kind="ExternalInput"))
    d_gx = nc.dram_tensor("gx", [NLOC, ND], f32, kind="ExternalOutput")

    with tile.TileContext(nc) as tc:
        with (
            tc.tile_pool(name="const", bufs=1) as constp,
            tc.tile_pool(name="idx", bufs=1) as idxp,
            tc.tile_pool(name="resid", bufs=1) as residp,
            tc.tile_pool(name="chunk", bufs=3) as chunkp,
            tc.tile_pool(name="win", bufs=2) as winp,
            tc.tile_pool(name="a1w", bufs=4) as a1wp,
            tc.tile_pool(name="pnd", bufs=2, space="PSUM") as pndp,
            tc.tile_pool(name="pa", bufs=4, space="PSUM") as pap,
            tc.tile_pool(name="pmisc", bufs=2, space="PSUM") as pmiscp,
            tc.tile_pool(name="dram", bufs=2, space="DRAM") as dramp,
        ):
            t_iota = constp.tile([128, 128], bf16)
            t_ident = constp.tile([128, 128], f32)
            t_identb = constp.tile([128, 128], bf16)
            t_W = constp.tile([128, 6, 384], bf16)
            t_bt = constp.tile([128, 6, 384], f32)
            nc.sync.dma_start(t_iota[:], d_iota[:])
            nc.sync.dma_start(t_ident[:], d_ident[:])
            nc.sync.dma_start(t_identb[:], d_identb[:])
            for l in range(6):
                nc.sync.dma_start(t_W[:, l, :], d_W[l])
                nc.sync.dma_start(t_bt[:, l, :], d_bt[l])

            t_srcw, t_slot = [], []
            for s in range(3):
                NT = NT_sets[s]
                a = idxp.tile([128, NT * 8], i16, tag=f"srcw{s}")
                c_ = idxp.tile([128, NT], f32, tag=f"slot{s}")
                nc.sync.dma_start(a[:], d_srcw[s][:])
                nc.sync.dma_start(c_[:], d_slot[s][:])
                t_srcw.append(a)
                t_slot.append(c_)

            t_g0 = residp.tile([128, NW, 128], bf16, tag="g0")
            t_g3 = residp.tile([128, NW, 128], bf16, tag="g3")

            tab_prev = None
            a1_prev = None

            last = NLAYERS - 1
            for l in range(NLAYERS):
                s = LAYER_SET[l]
                NT = NT_sets[s]
                T_w = T_w_sets[s]
                nchunks = (NT + C - 1) // C

                if l == 0:
                    tab_ap = d_tab0[:]
                    a1_dram = d_a1t0
                else:
                    tab_ap = tab_prev[:].rearrange("r n c -> (r n) c")
                    a1_dram = a1_prev

                if l < last:
                    b_agin = dramp.tile([NLOC, 256], bf16, tag="agin")
                    b_agout = dramp.tile([NCORES, NLOC, 256], bf16,
                                         tag="agout", addr_space="Shared")
                    b_a1n = dramp.tile([NLOCP, ND], bf16, tag="a1n")

                chunk_tiles = [None] * nchunks
                tile_win = []
                for w in range(NW):
                    tile_win += [w] * int(T_w[w])
                a1w_tiles = {}

                def get_a1w(w, a1_dram=a1_dram):
                    if w not in a1w_tiles:
                        t = a1wp.tile([128, 128], bf16, tag="t_a1w")
                        nc.sync.dma_start(t[:],
                                          a1_dram[w * 128:w * 128 + 128, :])
                        a1w_tiles[w] = t
                    return a1w_tiles[w]

                def make_chunk(k, s=s, NT=NT, tab_ap=tab_ap):
                    ck = min(C, NT - k * C)
                    ne = ck * 128
                    g_eh = chunkp.tile([128, C, 256], bf16, tag="g_eh")
                    t_St = chunkp.tile([128, C, 128], bf16, tag="t_St")
                    t_S = chunkp.tile([128, C, 128], bf16, tag="t_S")
                    t_e1 = chunkp.tile([128, C, 128], bf16, tag="t_e1")
                    t_e2 = chunkp.tile([128, C, 128], bf16, tag="t_e2")
                    t_rhs = chunkp.tile([128, C, 256], bf16, tag="t_rhs")
                    isl = slice(k * C * 8, (k * C + ck) * 8)
                    nc.gpsimd.dma_gather(g_eh[:, :ck, :], tab_ap,
                                         t_srcw[s][:, isl], ne, ne, 256,
                                         single_packet=False)
                    nc.sync.dma_start(t_St[:, :ck, :],
                                      d_St[s][:, k * C * 128:
                                              (k * C + ck) * 128]
                                      .rearrange("p (t e) -> p t e", e=128))
                    # S[e, tile, j] = (iota[e, j] == slot[e, tile])
                    nc.vector.tensor_tensor(
                        t_S[:, :ck, :],
                        t_iota[:, None, :].broadcast_to([128, ck, 128]),
                        t_slot[s][:, k * C:k * C + ck, None]
                        .broadcast_to([128, ck, 128]),
                        mybir.AluOpType.is_equal)
                    for j in range(ck):
                        # pre-activation a = St.T @ a1_win + a2 (PSUM)
                        ta1w = get_a1w(tile_win[k * C + j])
                        p_a = pap.tile([128, 128], f32, tag="p_a")
                        nc.tensor.matmul(p_a[:], t_St[:, j, :], ta1w[:],
                                         start=True, stop=False)
                        nc.tensor.matmul(p_a[:], t_identb[:],
                                         g_eh[:, j, 0:128],
                                         start=False, stop=True)
                        # t = exp(lrelu(a)) = max(exp(a), exp(0.2*a)):
                        # exp-only ACT usage -> no activation-table thrash
                        nc.scalar.activation(t_e1[:, j, :], p_a[:],
                                             mybir.ActivationFunctionType.Exp)
                        nc.scalar.activation(t_e2[:, j, :], p_a[:],
                                             mybir.ActivationFunctionType.Exp,
                                             scale=0.2)
                    nc.vector.tensor_tensor(t_rhs[:, :ck, 128:256],
                                            t_e1[:, :ck, :],
                                            t_e2[:, :ck, :],
                                            mybir.AluOpType.max)
                    nc.vector.tensor_tensor(t_rhs[:, :ck, 0:128],
                                            t_rhs[:, :ck, 128:256],
                                            g_eh[:, :ck, 128:256],
                                            mybir.AluOpType.mult)
                    return g_eh, t_St, t_S, t_e1, t_e2, t_rhs

                tg = 0
                for w in range(NW):
                    wl = WLEN[w]
                    p_nd = pndp.tile([128, 256], f32, tag="p_nd")
                    for i in range(int(T_w[w])):
                        k, j = divmod(tg, C)
                        if chunk_tiles[k] is None:
                            chunk_tiles[k] = make_chunk(k)
                        g_eh, t_St, t_S, t_e1, t_e2, t_rhs = chunk_tiles[k]
                        nc.tensor.matmul(p_nd[:], t_S[:, j, :], t_rhs[:, j, :],
                                         start=(i == 0),
                                         stop=(i == int(T_w[w]) - 1))
                        tg += 1

                    # window epilogue: x' = num * recip(den+eps) (+ residual)
                    t_den = winp.tile([128, 128], f32, tag="t_den")
                    t_rw = winp.tile([128, 128], f32, tag="t_rw")
                    t_xw = winp.tile([128, 128], f32, tag="t_xw")
                    nc.vector.tensor_scalar(t_den[:], p_nd[:, 128:256], 1e-16,
                                            None, mybir.AluOpType.add)
                    nc.vector.reciprocal(t_rw[:], t_den[:])
                    if l == 3:
                        t_x0 = winp.tile([128, 128], f32, tag="t_x0")
                        nc.vector.tensor_tensor(t_x0[:], p_nd[:, 0:128],
                                                t_rw[:], mybir.AluOpType.mult)
                        nc.vector.tensor_tensor(t_xw[:], t_x0[:],
                                                t_g0[:, w, :],
                                                mybir.AluOpType.add)
                    elif l == 6:
                        t_x0 = winp.tile([128, 128], f32, tag="t_x0")
                        nc.vector.tensor_tensor(t_x0[:], p_nd[:, 0:128],
                                                t_rw[:], mybir.AluOpType.mult)
                        nc.vector.tensor_tensor(t_xw[:], t_x0[:],
                                                t_g3[:, w, :],
                                                mybir.AluOpType.add)
                    else:
                        nc.vector.tensor_tensor(t_xw[:], p_nd[:, 0:128],
                                                t_rw[:], mybir.AluOpType.mult)
                    if l == 0:
                        nc.vector.tensor_copy(t_g0[:, w, :], t_xw[:])
                    if l == 3:
                        nc.vector.tensor_copy(t_g3[:, w, :], t_xw[:])

                    if l == last:
                        nc.sync.dma_start(d_gx[w * 128:w * 128 + wl, :],
                                          t_xw[:wl, :])
                    else:
                        # transpose -> table matmul -> bias/split -> bounce
                        p_tr = pmiscp.tile([128, 384], f32, tag="pm")
                        nc.tensor.transpose(p_tr[:, 0:128], t_xw[:],
                                            t_ident[:])
                        t_xT = winp.tile([128, 128], bf16, tag="t_xT")
                        nc.vector.tensor_copy(t_xT[:], p_tr[:, 0:128])
                        p_tab = pmiscp.tile([128, 384], f32, tag="pm")
                        nc.tensor.matmul(p_tab[:], t_xT[:], t_W[:, l, :],
                                         start=True, stop=True)
                        t_tb = winp.tile([128, 256], bf16, tag="t_tb")
                        t_a1o = winp.tile([128, 128], bf16, tag="t_a1o")
                        nc.vector.tensor_tensor(t_tb[:], p_tab[:, 0:256],
                                                t_bt[:, l, 0:256],
                                                mybir.AluOpType.add)
                        nc.vector.tensor_tensor(t_a1o[:], p_tab[:, 256:384],
                                                t_bt[:, l, 256:384],
                                                mybir.AluOpType.add)
                        nc.sync.dma_start(b_agin[w * 128:w * 128 + wl, :],
                                          t_tb[:wl, :])
                        nc.sync.dma_start(b_a1n[w * 128:w * 128 + 128, :],
                                          t_a1o[:])

                if l < last:
                    nc.gpsimd.collective_compute(
                        "AllGather", mybir.AluOpType.bypass,
                        replica_groups=[list(range(NCORES))],
                        ins=[b_agin[:].opt()], outs=[b_agout[:].opt()])
                    tab_prev = b_agout
                    a1_prev = b_a1n

    nc.compile()
    return nc


# ------------------------------------------------------------------ kernel

def kernel(**inputs):
    f32 = np.float32
    x = np.asarray(inputs["x"], f32)
    state_ = np.asarray(inputs["state_"], f32)
    input_ = np.asarray(inputs["input_"], f32)

    par = {k: np.asarray(v, f32) for k, v in inputs.items()
           if k not in ("x", "state_", "input_", "e_n", "e_r0", "e_r1",
                        "n_nodes")}

    sets_raw = [inputs["e_n"], inputs["e_r1"], inputs["e_r0"]]
    prep = [_prep_edge_set(np.asarray(e)) for e in sets_raw]
    T_w_sets = [p[1] for p in prep]
    NT_sets = [p[2] for p in prep]

    folds = []
    for l in range(7):
        p = LAYER_PAR[l]
        folds.append(_fold(par[f"{p}_lw"], par[f"{p}_lb"],
                           par[f"{p}_aw"], par[f"{p}_ab"]))

    W_dev = np.zeros((6, 128, 384), f32)
    bt_dev = np.zeros((6, 128, 384), f32)
    for l in range(1, 7):
        Wh, Wa1, Wa2, bh, b1, b2 = folds[l]
        W_dev[l - 1] = np.concatenate([Wa2, Wh, Wa1], axis=1)
        bt_dev[l - 1] = np.tile(np.concatenate([b2, bh, b1])[None, :],
                                (128, 1))

    Wh, Wa1, Wa2, bh, b1, b2 = folds[0]
    tab0 = np.concatenate([x @ Wa2 + b2, x @ Wh + bh], axis=1).astype(BF16)
    a1_0 = (x @ Wa1 + b1).astype(BF16)

    iota = np.tile(np.arange(128, dtype=f32), (128, 1)).astype(BF16)
    ident = np.eye(128, dtype=f32)

    meta_key = (NLAYERS, LRELU_VIA_DVE) + tuple(tuple(t) for t in T_w_sets)
    if meta_key not in _BUILD_CACHE:
        _BUILD_CACHE.clear()
        _BUILD_CACHE[meta_key] = _build(T_w_sets, NT_sets)
    nc = _BUILD_CACHE[meta_key]

    in_maps = []
    for c in range(NCORES):
        a1c = np.zeros((NLOCP, ND), BF16)
        a1c[:NLOC] = a1_0[c * NLOC:(c + 1) * NLOC]
        m = {
            "tab0": tab0,
            "a1t0": a1c,
            "iota": iota,
            "ident": ident,
            "identb": ident.astype(BF16),
            "W": W_dev.astype(BF16),
            "bt": bt_dev,
        }
        for s in range(3):
            src_ids, slot = prep[s][0][c]
            m[f"srcw{s}"] = _wrap_idx(src_ids)
            m[f"slot{s}"] = _slot_T(slot, NT_sets[s])
            m[f"St{s}"] = _make_St(slot, NT_sets[s])
        in_maps.append(m)

    res = run_bass_kernel_spmd(nc, in_maps, core_ids=list(range(NCORES)),
                               trace=TRACE)
    kernel.last_result = res
    gx = np.concatenate([res.results[c]["gx"] for c in range(NCORES)], axis=0)

    # ---- host postprocessing: heads ----
    xt = gx.reshape(B, NN, ND)[:, 1:]
    scores = (xt @ par["pw"].T + par["pb"])[..., 0]
    sm = np.exp(scores - scores.max(-1, keepdims=True))
    probs = (sm / sm.sum(-1, keepdims=True)).astype(f32)

    def elu(v):
        return np.where(v > 0, v, np.expm1(np.minimum(v, 0.0))).astype(f32)

    a_out = elu((xt @ par["mw"].T + par["mb"])[..., 0]) + 2
    b_out = elu(np.abs((xt @ par["sw"].T + par["sb"])[..., 0])) + 2

    # ---- host GRU ----
    st = state_.reshape(-1, ND)
    ii = (input_.reshape(st.shape[0], -1, EMB) @ par["win"].T).mean(1)
    si = np.concatenate([st, ii], -1)

    def sig(v):
        return 1.0 / (1.0 + np.exp(-v))

    z = sig(si @ par["wz"].T + par["bz"])
    r = sig(si @ par["wr"].T + par["br"])
    hc = np.tanh(np.concatenate([r * st, ii], -1) @ par["wh"].T + par["bh"])
    h = ((1 - z) * st + z * hc).astype(f32)

    return h, probs, a_out.astype(f32), b_out.astype(f32)


# revision 26
# speedup vs baseline: 1.4592x; 1.0400x over previous
"""Trainium2 Bass kernel for nn_Net_Actor (7-layer GAT stack + GRU head).

Sharding: 8 cores, core c owns dst nodes [4000c, 4000c+4000). Edges (incl.
self-loops) are dst-sorted per core and cut into 128-edge tiles aligned to
128-dst windows (tile counts equalized across cores so the SPMD program is
identical). Per edge tile:
  - a2|h rows (bf16, 512B) are fetched with one dma_gather (src-indexed)
  - a1[dst] is expanded from the window's contiguous a1 rows by a one-hot
    matmul (St, host-built), and a2 is added via an identity matmul into the
    same PSUM tile -> pre-activation a
  - t = exp(lrelu(a)) (no max subtraction; logits are small)
  - num|den segment sums via a one-hot matmul (S, built on-device) into a
    per-window PSUM accumulator
Between layers each core builds its table shard with one matmul per window
(folded weights) and an AllGather replicates the table. The GRU branch and
output heads run on host (0.02% of FLOPs).
"""
import sys

sys.path.insert(0, "/opt/trn_rl_repo")

import numpy as np
import ml_dtypes

import concourse.bacc as bacc
import concourse.mybir as mybir
import concourse.tile as tile
from concourse.bass_utils import run_bass_kernel_spmd

BF16 = ml_dtypes.bfloat16

NCORES = 8
ND = 128
N = 32000
NLOC = N // NCORES          # 4000
NLOCP = 4096                # padded a1 table rows
NW = (NLOC + 127) // 128    # 32 windows; last one covers 32 dst
WLEN = [128] * (NW - 1) + [NLOC - 128 * (NW - 1)]
PARTS = 4                   # AllGather split (8 windows each)
PSIZES = [1024, 1024, 1024, 928]     # rows per part per core
POFF = [0, 1024, 2048, 3072]         # row offset of part within a shard
GOFF = [0, 8192, 16384, 24576]       # row offset of part in the global table
B = 64
NN = 500
EMB = 128
C = 16                      # tiles per gather chunk
LRELU_VIA_DVE = False       # sim-compatible lrelu (max(x, 0.2x)); HW uses ACT
NLAYERS = 7                 # debug knob: run only the first NLAYERS layers
TRACE = False               # capture NTFF profile (sets exec_time_ns)

LAYER_SET = [0, 1, 1, 0, 2, 2, 0]
LAYER_PAR = ["g0", "g1", "g1", "gn", "g2", "g2", "gn"]


# ---------------------------------------------------------------- host prep

def _prep_edge_set(ei):
    """ei: [2, E'] int array (global). Per-core dst-sorted tile streams with
    window tile counts T_w equalized across cores. Self-loops are handled
    separately (contiguous rows), so they are NOT added here. Returns per
    core: (src_ids int16 (remapped), slot f32 (-1 padding)), plus T_w, NT."""
    src = np.asarray(ei[0], np.int64)
    dst = np.asarray(ei[1], np.int64)
    order = np.argsort(dst, kind="stable")
    src, dst = src[order], dst[order]

    lo = np.searchsorted(dst, np.arange(NCORES) * NLOC)
    hi = np.searchsorted(dst, (np.arange(NCORES) + 1) * NLOC)

    counts = np.zeros((NCORES, NW), np.int64)
    per_core = []
    for c in range(NCORES):
        s_c = src[lo[c]:hi[c]]
        d_c = dst[lo[c]:hi[c]] - c * NLOC
        counts[c] = np.bincount(d_c >> 7, minlength=NW)
        per_core.append((s_c, d_c))

    T_w = np.maximum(1, (counts.max(axis=0) + 127) // 128)
    NT = int(T_w.sum())

    out = []
    for c in range(NCORES):
        s_c, d_c = per_core[c]
        src_ids = np.zeros(NT * 128, np.int16)
        slot = np.full(NT * 128, -1.0, np.float32)
        off_e = 0
        off_t = 0
        for w in range(NW):
            ne = int(counts[c, w])
            sl = slice(off_t * 128, off_t * 128 + ne)
            src_ids[sl] = _remap_rows(s_c[off_e:off_e + ne])
            slot[sl] = (d_c[off_e:off_e + ne] - 128 * w).astype(np.float32)
            off_e += ne
            off_t += int(T_w[w])
        out.append((src_ids, slot))
    return out, T_w.astype(np.int64), NT


def _remap_rows(g):
    """global node id -> table row (rank-major AllGather concat = identity)."""
    return g


def _wrap_idx(idx):
    """[NT*128] -> [128, NT*8] int16: idx i at [i%16, i//16], replicated x8."""
    w = idx.reshape(-1, 16).T
    return np.tile(w, (8, 1)).astype(np.int16)


def _slot_T(slot, NT):
    """[NT*128] -> [128, NT]: edge e=(t*128+p) -> [p, t]."""
    return slot.reshape(NT, 128).T.copy()


def _make_St(slot, NT):
    """Transposed one-hots: St[j, t*128+e] = (slot[t*128+e] == j), bf16."""
    sl = slot.reshape(NT * 128).astype(np.int32)
    St = np.zeros((128, NT * 128), BF16)
    e = np.arange(NT * 128)
    m = sl >= 0
    St[sl[m], e[m]] = 1.0
    return St


def _fold(lw, lb, aw, ab):
    awi, awj = aw[:, :ND], aw[:, ND:]
    Wh = lw.T
    Wa1 = lw.T @ awi.T
    Wa2 = lw.T @ awj.T
    b1 = lb @ awi.T + ab
    b2 = lb @ awj.T
    return Wh, Wa1, Wa2, lb, b1, b2


# ------------------------------------------------------------- bass builder

_BUILD_CACHE = {}


def _build(T_w_sets, NT_sets):
    nc = bacc.Bacc("TRN2", target_bir_lowering=False, debug=False,
                   num_devices=NCORES)
    f32, bf16, i16 = mybir.dt.float32, mybir.dt.bfloat16, mybir.dt.int16

    d_tab0 = nc.dram_tensor("tab0", [N, 256], bf16, kind="ExternalInput")
    d_tab0s = nc.dram_tensor("tab0s", [NLOCP, 256], bf16, kind="ExternalInput")
    d_a1t0 = nc.dram_tensor("a1t0", [NLOCP, ND], bf16, kind="ExternalInput")
    d_iota = nc.dram_tensor("iota", [128, 128], bf16, kind="ExternalInput")
    d_ident = nc.dram_tensor("ident", [128, 128], f32, kind="ExternalInput")
    d_identb = nc.dram_tensor("identb", [128, 128], bf16, kind="ExternalInput")
    d_W = nc.dram_tensor("W", [6, 128, 384], bf16, kind="ExternalInput")
    d_bt = nc.dram_tensor("bt", [6, 128, 384], f32, kind="ExternalInput")
    d_srcw, d_slot, d_St = [], [], []
    for s in range(3):
        NT = NT_sets[s]
        d_srcw.append(nc.dram_tensor(f"srcw{s}", [128, NT * 8], i16,
                                     kind="ExternalInput"))
        d_slot.append(nc.dram_tensor(f"slot{s}", [128, NT], f32,
                                     kind="ExternalInput"))
        d_St.append(nc.dram_tensor(f"St{s}", [128, NT * 128], bf16,
                                   kind="ExternalInput"))
    d_gx = nc.dram_tensor("gx", [NLOC, ND], f32, kind="ExternalOutput")

    with tile.TileContext(nc) as tc:
        with (
            tc.tile_pool(name="const", bufs=1) as constp,
            tc.tile_pool(name="idx", bufs=1) as idxp,
            tc.tile_pool(name="resid", bufs=1) as residp,
            tc.tile_pool(name="chunk", bufs=3) as chunkp,
            tc.tile_pool(name="win", bufs=2) as winp,
            tc.tile_pool(name="a1w", bufs=4) as a1wp,
            tc.tile_pool(name="pnd", bufs=2, space="PSUM") as pndp,
            tc.tile_pool(name="pa", bufs=4, space="PSUM") as pap,
            tc.tile_pool(name="pmisc", bufs=2, space="PSUM") as pmiscp,
            tc.tile_pool(name="dram", bufs=2, space="DRAM") as dramp,
        ):
            t_iota = constp.tile([128, 128], bf16)
            t_ident = constp.tile([128, 128], f32)
            t_identb = constp.tile([128, 128], bf16)
            t_W = constp.tile([128, 6, 384], bf16)
            t_bt = constp.tile([128, 6, 384], f32)
            nc.sync.dma_start(t_iota[:], d_iota[:])
            nc.sync.dma_start(t_ident[:], d_ident[:])
            nc.sync.dma_start(t_identb[:], d_identb[:])
            for l in range(6):
                nc.sync.dma_start(t_W[:, l, :], d_W[l])
                nc.sync.dma_start(t_bt[:, l, :], d_bt[l])

            t_srcw, t_slot = [], []
            for s in range(3):
                NT = NT_sets[s]
                a = idxp.tile([128, NT * 8], i16, tag=f"srcw{s}")
                c_ = idxp.tile([128, NT], f32, tag=f"slot{s}")
                nc.sync.dma_start(a[:], d_srcw[s][:])
                nc.sync.dma_start(c_[:], d_slot[s][:])
                t_srcw.append(a)
                t_slot.append(c_)

            t_g0 = residp.tile([128, NW, 128], bf16, tag="g0")
            t_g3 = residp.tile([128, NW, 128], bf16, tag="g3")

            tab_prev = None
            a1_prev = None

            last = NLAYERS - 1
            for l in range(NLAYERS):
                s = LAYER_SET[l]
                NT = NT_sets[s]
                T_w = T_w_sets[s]
                nchunks = (NT + C - 1) // C

                if l == 0:
                    tab_ap = d_tab0[:]
                    self_dram = d_tab0s
                    a1_dram = d_a1t0
                else:
                    tab_ap = tab_prev[:]
                    self_dram = self_prev
                    a1_dram = a1_prev

                if l < last:
                    b_agin = dramp.tile([NLOC, 256], bf16, tag="agin")
                    b_agout = dramp.tile([N, 256], bf16,
                                         tag="agout", addr_space="Shared")
                    b_self = dramp.tile([NLOCP, 256], bf16, tag="self")
                    b_a1n = dramp.tile([NLOCP, ND], bf16, tag="a1n")

                chunk_tiles = [None] * nchunks
                tile_win = []
                for w in range(NW):
                    tile_win += [w] * int(T_w[w])
                a1w_tiles = {}

                def get_a1w(w, a1_dram=a1_dram):
                    if w not in a1w_tiles:
                        t = a1wp.tile([128, 128], bf16, tag="t_a1w")
                        nc.sync.dma_start(t[:],
                                          a1_dram[w * 128:w * 128 + 128, :])
                        a1w_tiles[w] = t
                    return a1w_tiles[w]

                def make_chunk(k, s=s, NT=NT, tab_ap=tab_ap):
                    ck = min(C, NT - k * C)
                    ne = ck * 128
                    g_eh = chunkp.tile([128, C, 256], bf16, tag="g_eh")
                    t_St = chunkp.tile([128, C, 128], bf16, tag="t_St")
                    t_S = chunkp.tile([128, C, 128], bf16, tag="t_S")
                    t_e1 = chunkp.tile([128, C, 128], bf16, tag="t_e1")
                    t_e2 = chunkp.tile([128, C, 128], bf16, tag="t_e2")
                    t_rhs = chunkp.tile([128, C, 256], bf16, tag="t_rhs")
                    isl = slice(k * C * 8, (k * C + ck) * 8)
                    nc.gpsimd.dma_gather(g_eh[:, :ck, :], tab_ap,
                                         t_srcw[s][:, isl], ne, ne, 256,
                                         single_packet=False)
                    nc.sync.dma_start(t_St[:, :ck, :],
                                      d_St[s][:, k * C * 128:
                                              (k * C + ck) * 128]
                                      .rearrange("p (t e) -> p t e", e=128))
                    # S[e, tile, j] = (iota[e, j] == slot[e, tile])
                    nc.vector.tensor_tensor(
                        t_S[:, :ck, :],
                        t_iota[:, None, :].broadcast_to([128, ck, 128]),
                        t_slot[s][:, k * C:k * C + ck, None]
                        .broadcast_to([128, ck, 128]),
                        mybir.AluOpType.is_equal)
                    for j in range(ck):
                        # pre-activation a = St.T @ a1_win + a2 (PSUM)
                        ta1w = get_a1w(tile_win[k * C + j])
                        p_a = pap.tile([128, 128], f32, tag="p_a")
                        nc.tensor.matmul(p_a[:], t_St[:, j, :], ta1w[:],
                                         start=True, stop=False)
                        nc.tensor.matmul(p_a[:], t_identb[:],
                                         g_eh[:, j, 0:128],
                                         start=False, stop=True)
                        # t = exp(lrelu(a)) = max(exp(a), exp(0.2*a)):
                        # exp-only ACT usage -> no activation-table thrash
                        nc.scalar.activation(t_e1[:, j, :], p_a[:],
                                             mybir.ActivationFunctionType.Exp)
                        nc.scalar.activation(t_e2[:, j, :], p_a[:],
                                             mybir.ActivationFunctionType.Exp,
                                             scale=0.2)
                    nc.vector.tensor_tensor(t_rhs[:, :ck, 128:256],
                                            t_e1[:, :ck, :],
                                            t_e2[:, :ck, :],
                                            mybir.AluOpType.max)
                    nc.vector.tensor_tensor(t_rhs[:, :ck, 0:128],
                                            t_rhs[:, :ck, 128:256],
                                            g_eh[:, :ck, 128:256],
                                            mybir.AluOpType.mult)
                    return g_eh, t_St, t_S, t_e1, t_e2, t_rhs

                tg = 0
                for w in range(NW):
                    wl = WLEN[w]
                    p_nd = pndp.tile([128, 256], f32, tag="p_nd")
                    # self-loop contribution from contiguous own-shard rows
                    ta1w = get_a1w(w)
                    t_ehw = winp.tile([128, 256], bf16, tag="t_ehw")
                    t_asf = winp.tile([128, 128], f32, tag="t_asf")
                    t_es1 = winp.tile([128, 128], bf16, tag="t_es1")
                    t_es2 = winp.tile([128, 128], bf16, tag="t_es2")
                    t_rsf = winp.tile([128, 256], bf16, tag="t_rsf")
                    nc.sync.dma_start(t_ehw[:wl, :],
                                      self_dram[w * 128:w * 128 + wl, :])
                    nc.vector.tensor_tensor(t_asf[:wl, :], ta1w[:wl, :],
                                            t_ehw[:wl, 0:128],
                                            mybir.AluOpType.add)
                    nc.scalar.activation(t_es1[:wl, :], t_asf[:wl, :],
                                         mybir.ActivationFunctionType.Exp)
                    nc.scalar.activation(t_es2[:wl, :], t_asf[:wl, :],
                                         mybir.ActivationFunctionType.Exp,
                                         scale=0.2)
                    if wl < 128:
                        nc.vector.memset(t_rsf[:, :], 0.0)
                    nc.vector.tensor_tensor(t_rsf[:wl, 128:256],
                                            t_es1[:wl, :], t_es2[:wl, :],
                                            mybir.AluOpType.max)
                    nc.vector.tensor_tensor(t_rsf[:wl, 0:128],
                                            t_rsf[:wl, 128:256],
                                            t_ehw[:wl, 128:256],
                                            mybir.AluOpType.mult)
                    nc.tensor.matmul(p_nd[:], t_identb[:], t_rsf[:],
                                     start=True, stop=False)
                    for i in range(int(T_w[w])):
                        k, j = divmod(tg, C)
                        if chunk_tiles[k] is None:
                            chunk_tiles[k] = make_chunk(k)
                        g_eh, t_St, t_S, t_e1, t_e2, t_rhs = chunk_tiles[k]
                        nc.tensor.matmul(p_nd[:], t_S[:, j, :], t_rhs[:, j, :],
                                         start=False,
                                         stop=(i == int(T_w[w]) - 1))
                        tg += 1

                    # window epilogue: x' = num * recip(den+eps) (+ residual)
                    t_den = winp.tile([128, 128], f32, tag="t_den")
                    t_rw = winp.tile([128, 128], f32, tag="t_rw")
                    t_xw = winp.tile([128, 128], f32, tag="t_xw")
                    nc.vector.tensor_scalar(t_den[:], p_nd[:, 128:256], 1e-16,
                                            None, mybir.AluOpType.add)
                    nc.vector.reciprocal(t_rw[:], t_den[:])
                    if l == 3:
                        t_x0 = winp.tile([128, 128], f32, tag="t_x0")
                        nc.vector.tensor_tensor(t_x0[:], p_nd[:, 0:128],
                                                t_rw[:], mybir.AluOpType.mult)
                        nc.vector.tensor_tensor(t_xw[:], t_x0[:],
                                                t_g0[:, w, :],
                                                mybir.AluOpType.add)
                    elif l == 6:
                        t_x0 = winp.tile([128, 128], f32, tag="t_x0")
                        nc.vector.tensor_tensor(t_x0[:], p_nd[:, 0:128],
                                                t_rw[:], mybir.AluOpType.mult)
                        nc.vector.tensor_tensor(t_xw[:], t_x0[:],
                                                t_g3[:, w, :],
                                                mybir.AluOpType.add)
                    else:
                        nc.vector.tensor_tensor(t_xw[:], p_nd[:, 0:128],
                                                t_rw[:], mybir.AluOpType.mult)
                    if l == 0:
                        nc.vector.tensor_copy(t_g0[:, w, :], t_xw[:])
                    if l == 3:
                        nc.vector.tensor_copy(t_g3[:, w, :], t_xw[:])

                    if l == last:
                        nc.sync.dma_start(d_gx[w * 128:w * 128 + wl, :],
                                          t_xw[:wl, :])
                    else:
                        # transpose -> table matmul -> bias/split -> bounce
                        p_tr = pmiscp.tile([128, 384], f32, tag="pm")
                        nc.tensor.transpose(p_tr[:, 0:128], t_xw[:],
                                            t_ident[:])
                        t_xT = winp.tile([128, 128], bf16, tag="t_xT")
                        nc.vector.tensor_copy(t_xT[:], p_tr[:, 0:128])
                        p_tab = pmiscp.tile([128, 384], f32, tag="pm")
                        nc.tensor.matmul(p_tab[:], t_xT[:], t_W[:, l, :],
                                         start=True, stop=True)
                        t_tb = winp.tile([128, 256], bf16, tag="t_tb")
                        t_a1o = winp.tile([128, 128], bf16, tag="t_a1o")
                        nc.vector.tensor_tensor(t_tb[:], p_tab[:, 0:256],
                                                t_bt[:, l, 0:256],
                                                mybir.AluOpType.add)
                        nc.vector.tensor_tensor(t_a1o[:], p_tab[:, 256:384],
                                                t_bt[:, l, 256:384],
                                                mybir.AluOpType.add)
                        nc.sync.dma_start(b_agin[w * 128:w * 128 + wl, :],
                                          t_tb[:wl, :])
                        nc.sync.dma_start(b_self[w * 128:w * 128 + wl, :],
                                          t_tb[:wl, :])
                        nc.sync.dma_start(b_a1n[w * 128:w * 128 + 128, :],
                                          t_a1o[:])


                if l < last:
                    nc.gpsimd.collective_compute(
                        "AllGather", mybir.AluOpType.bypass,
                        replica_groups=[list(range(NCORES))],
                        ins=[b_agin[:].opt()], outs=[b_agout[:].opt()])
                    tab_prev = b_agout
                    self_prev = b_self
                    a1_prev = b_a1n

    nc.compile()
    return nc


# ------------------------------------------------------------------ kernel

def kernel(**inputs):
    f32 = np.float32
    x = np.asarray(inputs["x"], f32)
    state_ = np.asarray(inputs["state_"], f32)
    input_ = np.asarray(inputs["input_"], f32)

    par = {k: np.asarray(v, f32) for k, v in inputs.items()
           if k not in ("x", "state_", "input_", "e_n", "e_r0", "e_r1",
                        "n_nodes")}

    sets_raw = [inputs["e_n"], inputs["e_r1"], inputs["e_r0"]]
    prep = [_prep_edge_set(np.asarray(e)) for e in sets_raw]
    T_w_sets = [p[1] for p in prep]
    NT_sets = [p[2] for p in prep]

    folds = []
    for l in range(7):
        p = LAYER_PAR[l]
        folds.append(_fold(par[f"{p}_lw"], par[f"{p}_lb"],
                           par[f"{p}_aw"], par[f"{p}_ab"]))

    W_dev = np.zeros((6, 128, 384), f32)
    bt_dev = np.zeros((6, 128, 384), f32)
    for l in range(1, 7):
        Wh, Wa1, Wa2, bh, b1, b2 = folds[l]
        W_dev[l - 1] = np.concatenate([Wa2, Wh, Wa1], axis=1)
        bt_dev[l - 1] = np.tile(np.concatenate([b2, bh, b1])[None, :],
                                (128, 1))

    Wh, Wa1, Wa2, bh, b1, b2 = folds[0]
    tab0_orig = np.concatenate([x @ Wa2 + b2, x @ Wh + bh], axis=1).astype(BF16)
    tab0 = np.zeros_like(tab0_orig)
    tab0[_remap_rows(np.arange(N))] = tab0_orig
    a1_0 = (x @ Wa1 + b1).astype(BF16)

    iota = np.tile(np.arange(128, dtype=f32), (128, 1)).astype(BF16)
    ident = np.eye(128, dtype=f32)

    meta_key = (NLAYERS, LRELU_VIA_DVE) + tuple(tuple(t) for t in T_w_sets)
    if meta_key not in _BUILD_CACHE:
        _BUILD_CACHE.clear()
        _BUILD_CACHE[meta_key] = _build(T_w_sets, NT_sets)
    nc = _BUILD_CACHE[meta_key]

    in_maps = []
    for c in range(NCORES):
        a1c = np.zeros((NLOCP, ND), BF16)
        a1c[:NLOC] = a1_0[c * NLOC:(c + 1) * NLOC]
        t0s = np.zeros((NLOCP, 256), BF16)
        t0s[:NLOC] = tab0_orig[c * NLOC:(c + 1) * NLOC]
        m = {
            "tab0": tab0,
            "tab0s": t0s,
            "a1t0": a1c,
            "iota": iota,
            "ident": ident,
            "identb": ident.astype(BF16),
            "W": W_dev.astype(BF16),
            "bt": bt_dev,
        }
        for s in range(3):
            src_ids, slot = prep[s][0][c]
            m[f"srcw{s}"] = _wrap_idx(src_ids)
            m[f"slot{s}"] = _slot_T(slot, NT_sets[s])
            m[f"St{s}"] = _make_St(slot, NT_sets[s])
        in_maps.append(m)

    res = run_bass_kernel_spmd(nc, in_maps, core_ids=list(range(NCORES)),
                               trace=TRACE)
    kernel.last_result = res
    gx = np.concatenate([res.results[c]["gx"] for c in range(NCORES)], axis=0)

    # ---- host postprocessing: heads ----
    xt = gx.reshape(B, NN, ND)[:, 1:]
    scores = (xt @ par["pw"].T + par["pb"])[..., 0]
    sm = np.exp(scores - scores.max(-1, keepdims=True))
    probs = (sm / sm.sum(-1, keepdims=True)).astype(f32)

    def elu(v):
        return np.where(v > 0, v, np.expm1(np.minimum(v, 0.0))).astype(f32)

    a_out = elu((xt @ par["mw"].T + par["mb"])[..., 0]) + 2
    b_out = elu(np.abs((xt @ par["sw"].T + par["sb"])[..., 0])) + 2

    # ---- host GRU ----
    st = state_.reshape(-1, ND)
    ii = (input_.reshape(st.shape[0], -1, EMB) @ par["win"].T).mean(1)
    si = np.concatenate([st, ii], -1)

    def sig(v):
        return 1.0 / (1.0 + np.exp(-v))

    z = sig(si @ par["wz"].T + par["bz"])
    r = sig(si @ par["wr"].T + par["br"])
    hc = np.tanh(np.concatenate([r * st, ii], -1) @ par["wh"].T + par["bh"])
    h = ((1 - z) * st + z * hc).astype(f32)

    return h, probs, a_out.astype(f32), b_out.astype(f32)
